# revision 1
# baseline (speedup 1.0000x reference)
"""Expert-parallel MoE routing kernel for Trainium2 (8 NeuronCores).

Problem: out[t] = x[t] @ W[idx[t]].T + b[idx[t]],  idx = pointer_addresses % 8
  x: [2048, 512] f32, W: [8, 8192, 512] f32, b: [8, 8192] f32 -> out [2048, 8192] f32

Strategy: expert parallel. Host computes idx, gathers each expert's tokens
(padded to a common capacity `cap`), and each core e computes
  out_e = x_e @ W[e].T + b[e]
with the vocab dimension on PSUM partitions so the bias is a fused
per-partition bias on the Scalar/Vector engines. Host scatters rows back.

Per-core matmul orientation (out = lhsT.T @ rhs):
  lhsT = W chunk  [K=128 (d inner), M=128 (vocab cols)]   (stationary)
  rhs  = xT chunk [K=128 (d inner), N=cap (tokens)]        (moving)
  psum [128 vocab, cap tokens] accumulated over 4 K-chunks of D=512.

The 64 vocab chunks are streamed in groups; group sizes are graduated
(small first/last) so the serial prologue (first W load) and epilogue
(last out store) are short while steady-state DMAs stay large.
"""

import os

import numpy as np

E = 8          # experts == cores
D = 512        # hidden
V = 8192       # out features
P = 128        # partitions
KCH = D // P   # 4 contraction chunks
VCH = V // P   # 64 vocab chunks

# matmul input dtype: 'f32' (exact, 4 cyc/row), 'f32r' (~full speed, ~1.3e-4
# rel err), 'fp16'/'bf16' (full speed, ~2.6e-4 / ~2.1e-3 rel err)
MM_DTYPE = os.environ.get("KERNEL_MM_DTYPE", "fp16")
# output storage dtype: 'f32' (exact) | 'fp16' (~2.4e-4 quant err, halves out bytes)
OUT_DTYPE = os.environ.get("KERNEL_OUT_DTYPE", "fp16")
_GROUPS_ENV = os.environ.get("KERNEL_GROUPS")

LAST_RESULT = None  # BassKernelResults of the most recent run (for test harness)

_BUILD_CACHE = {}


def _in_sz():
    return 2 if MM_DTYPE in ("bf16", "fp16") else 4


def _out_sz():
    return 2 if OUT_DTYPE in ("bf16", "fp16") else 4


def _base_gv(cap):
    """Steady-state vocab chunks per DMA group: as large as SBUF allows.

    Per-partition slab budgets: w tiles gv*KCH*P*in_sz (x3 bufs), o tiles
    gv*cap*out_sz (x2 bufs), plus the resident x tile. 16 fits at the
    nominal cap (~274); shrink for pathologically imbalanced routing.
    """
    for gv in (16, 8, 4, 2, 1):
        if (
            gv * KCH * P * _in_sz() * 3
            + gv * cap * _out_sz() * 2
            + KCH * cap * _in_sz()
            <= 168 * 1024
        ):
            return gv
    return 1


def _groups(cap):
    """Graduated group schedule over the 64 vocab chunks."""
    if _GROUPS_ENV:
        sched = [int(v) for v in _GROUPS_ENV.split(",")]
        assert sum(sched) == VCH
        return sched
    base = _base_gv(cap)
    sched = []
    left = VCH
    while left > 0:
        g = min(base, left)
        sched.append(g)
        left -= g
    assert sum(sched) == VCH
    return sched


def _build(cap, repeat=1, loop_n=1):
    """Build the per-core Bass module for token capacity `cap`.

    repeat/loop_n > 1 re-run the compute loop (same outputs) so the test
    harness can difference wall-times to isolate on-device kernel time;
    loop_n uses a hardware For_i loop (constant code size).
    """
    key = (cap, MM_DTYPE, OUT_DTYPE, tuple(_groups(cap)), repeat, loop_n)
    if key in _BUILD_CACHE:
        return _BUILD_CACHE[key]

    import concourse.mybir as mybir
    from concourse import bacc
    from concourse.tile import TileContext

    dt_in = {
        "f32": mybir.dt.float32,
        "f32r": mybir.dt.float32r,
        "bf16": mybir.dt.bfloat16,
        "fp16": mybir.dt.float16,
    }[MM_DTYPE]
    f32 = mybir.dt.float32
    out_dt = {"f32": f32, "fp16": mybir.dt.float16, "bf16": mybir.dt.bfloat16}[
        OUT_DTYPE
    ]
    sched = _groups(cap)
    gmax = max(sched)

    nc = bacc.Bacc(None, target_bir_lowering=False)
    # flat layouts, vocab-chunk (vi) as the per-partition-contiguous axis
    wt = nc.dram_tensor("wt", [P, VCH, KCH, P], dt_in, kind="ExternalInput")
    xt = nc.dram_tensor("xt", [P, KCH, cap], dt_in, kind="ExternalInput")
    bias = nc.dram_tensor("bias", [P, VCH], f32, kind="ExternalInput")
    out = nc.dram_tensor("out", [P, VCH, cap], out_dt, kind="ExternalOutput")

    # token chunks of <=512 (PSUM bank / f32 moving-operand limit)
    t_chunks = []
    t0 = 0
    while t0 < cap:
        t_chunks.append((t0, min(512, cap - t0)))
        t0 += 512

    def group_body():
        vi0 = 0
        for nvi in sched:
            w_full = wp.tile([P, gmax, KCH, P], dt_in, tag="w", name="w_full")
            w_sb = w_full[:, :nvi]
            nc.sync.dma_start(w_sb, wt.ap()[:, vi0 : vi0 + nvi])
            o_full = op_.tile([P, gmax, cap], out_dt, tag="o", name="o_full")
            o_sb = o_full[:, :nvi]
            for g in range(nvi):
                vi = vi0 + g
                for tc0, tw in t_chunks:
                    ps = pp.tile([P, tw], f32, tag="ps")
                    for k in range(KCH):
                        nc.tensor.matmul(
                            ps,
                            lhsT=w_sb[:, g, k],
                            rhs=x_sb[:, k, tc0 : tc0 + tw],
                            start=(k == 0),
                            stop=(k == KCH - 1),
                        )
                    # out = psum + bias; alternate ScalarE / VectorE so neither
                    # engine's eviction throughput becomes the bottleneck
                    if vi % 2 == 0:
                        nc.scalar.activation(
                            o_sb[:, g, tc0 : tc0 + tw],
                            ps,
                            mybir.ActivationFunctionType.Identity,
                            bias=b_sb[:, vi : vi + 1],
                            scale=1.0,
                        )
                    else:
                        nc.vector.tensor_tensor(
                            o_sb[:, g, tc0 : tc0 + tw],
                            ps,
                            b_sb[:, vi : vi + 1].to_broadcast((P, tw)),
                            mybir.AluOpType.add,
                        )
            # out DMAs ride the ACT HWDGE ring so they don't serialize
            # behind the W loads on the SP ring
            nc.scalar.dma_start(out.ap()[:, vi0 : vi0 + nvi], o_sb)
            vi0 += nvi

    with TileContext(nc) as tc:
        with (
            tc.tile_pool(name="xp", bufs=1) as xp,
            tc.tile_pool(name="bp", bufs=1) as bp,
            tc.tile_pool(name="wp", bufs=3) as wp,
            tc.tile_pool(name="op", bufs=2) as op_,
            tc.tile_pool(name="pp", bufs=8, space="PSUM") as pp,
        ):
            x_sb = xp.tile([P, KCH, cap], dt_in)
            nc.gpsimd.dma_start(x_sb, xt.ap())
            b_sb = bp.tile([P, VCH], f32)
            nc.gpsimd.dma_start(b_sb, bias.ap())

            import contextlib

            loop_cm = (
                tc.For_i(0, loop_n, 1) if loop_n > 1 else contextlib.nullcontext()
            )
            with loop_cm:
                for _rep in range(repeat):
                    group_body()

    nc.finalize()
    _BUILD_CACHE[key] = nc
    return nc


def _prepare(x, pointer_addresses, W, b):
    """Host-side shard: gather tokens per expert, lay out per-core inputs."""
    x = np.ascontiguousarray(np.asarray(x), dtype=np.float32)
    W = np.ascontiguousarray(np.asarray(W), dtype=np.float32)
    b = np.ascontiguousarray(np.asarray(b), dtype=np.float32)
    pa = np.asarray(pointer_addresses)

    idx = (pa.astype(np.int64) % E).astype(np.int64)
    rows = [np.flatnonzero(idx == e) for e in range(E)]
    counts = np.array([len(r) for r in rows])
    cap = max(256, int(counts.max()))

    if MM_DTYPE == "bf16":
        import ml_dtypes

        np_dt = np.dtype(ml_dtypes.bfloat16)
    elif MM_DTYPE == "fp16":
        np_dt = np.dtype(np.float16)
    else:
        np_dt = np.dtype(np.float32)

    in_maps = []
    for e in range(E):
        # xT: [P(d inner), KCH, cap]
        x_pad = np.zeros((cap, D), np.float32)
        x_pad[: counts[e]] = x[rows[e]]
        xt_e = np.ascontiguousarray(
            x_pad.reshape(cap, KCH, P).transpose(2, 1, 0).astype(np_dt)
        )
        # wt: [p, vi, k, c] = W[e, vi*P + c, k*P + p]
        w_e = np.ascontiguousarray(
            W[e].reshape(VCH, P, KCH, P).transpose(3, 0, 2, 1).astype(np_dt)
        )
        # bias: [P(c), VCH]
        b_e = np.ascontiguousarray(b[e].reshape(VCH, P).T)
        in_maps.append({"wt": w_e, "xt": xt_e, "bias": b_e})

    return in_maps, rows, counts, cap


def _run(nc, in_maps):
    global LAST_RESULT
    from concourse.bass_utils import run_bass_kernel_spmd

    res = run_bass_kernel_spmd(nc, in_maps, core_ids=list(range(E)))
    LAST_RESULT = res
    return res


def _assemble(res, rows, counts, cap, n_tokens):
    out = np.zeros((n_tokens, V), np.float32)
    for e in range(E):
        # out dram [P(c), VCH, cap] -> vocab-major [V, cap]
        o = (
            res.results[e]["out"]
            .astype(np.float32)
            .transpose(1, 0, 2)
            .reshape(V, cap)
        )
        out[rows[e]] = o[:, : counts[e]].T
    return out


def kernel(x, pointer_addresses, W, b):
    in_maps, rows, counts, cap = _prepare(x, pointer_addresses, W, b)
    nc = _build(cap)
    res = _run(nc, in_maps)
    return _assemble(res, rows, counts, cap, np.asarray(x).shape[0])



# revision 2
# speedup vs baseline: 2.8802x; 2.8802x over previous
"""Expert-parallel MoE routing kernel for Trainium2 (8 NeuronCores).

Problem: out[t] = x[t] @ W[idx[t]].T + b[idx[t]],  idx = pointer_addresses % 8
  x: [2048, 512] f32, W: [8, 8192, 512] f32, b: [8, 8192] f32 -> out [2048, 8192] f32

Sharding: expert-parallel. The host computes idx, gathers each expert's tokens
(padded to a common capacity `cap`), and core e computes
  out_e = x_e @ W[e].T + b[e]
with the vocab dim on PSUM partitions so the bias is a fused per-partition
bias on the Scalar/Vector engines. The host scatters rows back.

Per-core design (measured ~46 us/exec on 8 concurrent cores, vs a ~41 us
practical DMA roofline for the fp16 version of this traffic):

- W is quantized host-side to int8 (symmetric, per-expert scale
  q_e = max|W[e]|/127) and DMA'd with an int8->fp16 cast (SWDGE/gpsimd),
  HALVING the dominant HBM stream (8.4 MB -> 4.2 MB per core). The dequant
  scale is folded into the tokens: xt_e = x_e * q_e, so the NEFF is
  identical across cores (SPMD) and no on-device rescale is needed.
  Quantization error ~3.4e-3 max-rel (threshold 2e-2), since uniform W
  quantizes ~9x more accurately in int8 than fp8.
- Matmul orientation (out = lhsT.T @ rhs): lhsT = W chunk [K=128, M=128]
  stationary, rhs = xT chunk [K=128, N=cap] moving, PSUM [128 vocab, cap]
  accumulated over 4 K-chunks of D=512.
- W loads are grouped [4,12,16,16,16] vocab-chunks (small first so the PE
  stream starts ~2 us in, large after so the DMA ring stays efficient);
  out stores are grouped [16,16,16,12,4] (small last so the post-compute
  store tail is short). Groups are decoupled.
- Engine layout: W loads on gpsimd (SWDGE, cast), out stores on sync
  (SP HWDGE, otherwise idle), PSUM evictions alternate Scalar/Vector, with
  the tail group forced to Vector so the last store is not queued behind
  ACT work.
"""

import contextlib

import numpy as np

E = 8          # experts == cores
D = 512        # hidden
V = 8192       # out features
P = 128        # partitions
KCH = D // P   # 4 contraction chunks
VCH = V // P   # 64 vocab chunks

SCHED_W = (4, 12, 16, 16, 16)   # W-load groups (vocab chunks)
SCHED_O = (16, 16, 16, 12, 4)   # out-store groups
WP_BUFS = 4

LAST_RESULT = None  # BassKernelResults of the most recent run (for harness)

_BUILD_CACHE = {}


def _build(cap, loop_n=1):
    """Build the per-core Bass module for token capacity `cap`.

    loop_n > 1 wraps the compute in a hardware For_i re-running it (same
    outputs) so a test harness can difference wall-times to isolate the
    on-device per-execution time.
    """
    key = (cap, loop_n)
    if key in _BUILD_CACHE:
        return _BUILD_CACHE[key]

    import concourse.mybir as mybir
    from concourse import bacc
    from concourse.tile import TileContext

    i8 = mybir.dt.int8
    fp16 = mybir.dt.float16
    f32 = mybir.dt.float32
    gw = max(SCHED_W)
    go = max(SCHED_O)

    nc = bacc.Bacc(None, target_bir_lowering=False)
    # flat layouts, vocab-chunk (vi) as the per-partition-contiguous axis
    wt = nc.dram_tensor("wt", [P, VCH, KCH, P], i8, kind="ExternalInput")
    xt = nc.dram_tensor("xt", [P, KCH, cap], fp16, kind="ExternalInput")
    bias = nc.dram_tensor("bias", [P, VCH], f32, kind="ExternalInput")
    out = nc.dram_tensor("out", [P, VCH, cap], fp16, kind="ExternalOutput")

    # token chunks of <=512 (PSUM bank f32 limit)
    t_chunks = []
    t0 = 0
    while t0 < cap:
        t_chunks.append((t0, min(512, cap - t0)))
        t0 += 512

    w_start, o_start, o_end = {}, {}, {}
    s = 0
    for n in SCHED_W:
        w_start[s] = n
        s += n
    s = 0
    for n in SCHED_O:
        o_start[s] = n
        o_end[s + n] = (s, n)
        s += n

    def body():
        w_sb = o_sb = None
        w0 = o0 = 0
        for vi in range(VCH):
            if vi in w_start:
                nvi = w_start[vi]
                w_full = wp.tile([P, gw, KCH, P], fp16, tag="w", name="w_full")
                w_sb = w_full[:, :nvi]
                # int8 DRAM -> fp16 SBUF cast during the SWDGE DMA
                nc.gpsimd.dma_start(w_sb, wt.ap()[:, vi : vi + nvi])
                w0 = vi
            if vi in o_start:
                o_full = op_.tile([P, go, cap], fp16, tag="o", name="o_full")
                o_sb = o_full[:, : o_start[vi]]
                o0 = vi
            for tc0, tw in t_chunks:
                ps = pp.tile([P, tw], f32, tag="ps")
                for k in range(KCH):
                    nc.tensor.matmul(
                        ps,
                        lhsT=w_sb[:, vi - w0, k],
                        rhs=x_sb[:, k, tc0 : tc0 + tw],
                        start=(k == 0),
                        stop=(k == KCH - 1),
                    )
                # out = psum + bias; alternate ScalarE / VectorE so neither
                # engine's eviction throughput becomes the bottleneck; the
                # tail store-group goes all-Vector so the final SP store
                # isn't serialized behind ACT evictions
                if vi >= VCH - SCHED_O[-1] or vi % 2:
                    nc.vector.tensor_tensor(
                        o_sb[:, vi - o0, tc0 : tc0 + tw], ps,
                        b_sb[:, vi : vi + 1].to_broadcast((P, tw)),
                        mybir.AluOpType.add,
                    )
                else:
                    nc.scalar.activation(
                        o_sb[:, vi - o0, tc0 : tc0 + tw], ps,
                        mybir.ActivationFunctionType.Identity,
                        bias=b_sb[:, vi : vi + 1], scale=1.0,
                    )
            if vi + 1 in o_end:
                b0, n = o_end[vi + 1]
                nc.sync.dma_start(out.ap()[:, b0 : b0 + n], o_sb)

    with TileContext(nc) as tc:
        with (
            tc.tile_pool(name="xp", bufs=1) as xp,
            tc.tile_pool(name="bp", bufs=1) as bp,
            tc.tile_pool(name="wp", bufs=WP_BUFS) as wp,
            tc.tile_pool(name="op", bufs=2) as op_,
            tc.tile_pool(name="pp", bufs=8, space="PSUM") as pp,
        ):
            x_sb = xp.tile([P, KCH, cap], fp16)
            nc.gpsimd.dma_start(x_sb, xt.ap())
            b_sb = bp.tile([P, VCH], f32)
            nc.gpsimd.dma_start(b_sb, bias.ap())

            loop_cm = (
                tc.For_i(0, loop_n, 1) if loop_n > 1 else contextlib.nullcontext()
            )
            with loop_cm:
                body()

    nc.finalize()
    _BUILD_CACHE[key] = nc
    return nc


def _prepare(x, pointer_addresses, W, b):
    """Host-side shard: gather tokens per expert, quantize W, scale tokens."""
    x = np.ascontiguousarray(np.asarray(x), dtype=np.float32)
    W = np.ascontiguousarray(np.asarray(W), dtype=np.float32)
    b = np.ascontiguousarray(np.asarray(b), dtype=np.float32)
    pa = np.asarray(pointer_addresses)

    idx = (pa.astype(np.int64) % E).astype(np.int64)
    rows = [np.flatnonzero(idx == e) for e in range(E)]
    counts = np.array([len(r) for r in rows])
    cap = max(256, int(counts.max()))

    in_maps = []
    for e in range(E):
        q = float(np.abs(W[e]).max()) / 127.0
        if q == 0.0:
            q = 1.0
        wq = np.clip(np.round(W[e] / q), -127, 127).astype(np.int8)
        # wt: [p, vi, k, c] = Wq[vi*P + c, k*P + p]
        w_e = np.ascontiguousarray(
            wq.reshape(VCH, P, KCH, P).transpose(3, 0, 2, 1)
        )
        # xT: [P(d inner), KCH, cap], pre-scaled by the dequant factor q
        x_pad = np.zeros((cap, D), np.float32)
        x_pad[: counts[e]] = x[rows[e]] * q
        xt_e = np.ascontiguousarray(
            x_pad.reshape(cap, KCH, P).transpose(2, 1, 0).astype(np.float16)
        )
        # bias: [P(c), VCH]
        b_e = np.ascontiguousarray(b[e].reshape(VCH, P).T)
        in_maps.append({"wt": w_e, "xt": xt_e, "bias": b_e})

    return in_maps, rows, counts, cap


def _run(nc, in_maps):
    global LAST_RESULT
    from concourse.bass_utils import run_bass_kernel_spmd

    res = run_bass_kernel_spmd(nc, in_maps, core_ids=list(range(E)))
    LAST_RESULT = res
    return res


def _assemble(res, rows, counts, cap, n_tokens):
    out = np.zeros((n_tokens, V), np.float32)
    for e in range(E):
        # out dram [P(c), VCH, cap] -> vocab-major [V, cap]
        o = (
            res.results[e]["out"]
            .astype(np.float32)
            .transpose(1, 0, 2)
            .reshape(V, cap)
        )
        out[rows[e]] = o[:, : counts[e]].T
    return out


def kernel(x, pointer_addresses, W, b):
    in_maps, rows, counts, cap = _prepare(x, pointer_addresses, W, b)
    nc = _build(cap)
    res = _run(nc, in_maps)
    return _assemble(res, rows, counts, cap, np.asarray(x).shape[0])
